# revision 10
# baseline (speedup 1.0000x reference)
"""Trainium2 Bass kernel for nn_Encoder_Cross (sparse_attention).

Per batch element b (8 of them, one per NeuronCore):
  x [V=16, P=128, PD=2048]; attn1 per-variable over patches; attn2
  per-patch over variables (masked); FFN 2048->8192->2048 (exact gelu);
  residuals + 3 LayerNorms. Outputs (x_out, x_next=reshape(x_out)).

Sharding: pure data-parallel over B (B=8 == 8 cores), no collectives.

All matmuls run as float32r (full PE rate, ~tf32 rounding). One 16.8MB
SBUF region is reused through the phases: XT -> KVT -> X1T -> KV2T -> X2T;
large intermediates (KVT, KVo, h1, X1, X2) spill through HBM.
"""

import math
import time
from contextlib import ExitStack

import numpy as np

B, V, P, L, D = 8, 16, 128, 16, 128
PD = L * D            # 2048
T = V * P             # 2048 tokens per core
KT = PD // P          # 16 k tiles
NCH = 512             # moving chunk
NT = T // NCH         # 4 token chunks
H = 4 * PD            # 8192
HT = H // P           # 64 h tiles
GP = P // V           # 8 patches per group
NG = P // GP          # 16 groups
SCALE = 1.0 / math.sqrt(PD)
EPS = 1e-5
NEG = -1.0e30


def build_encoder(flags):
    import concourse.bass as bass
    import concourse.bacc as bacc
    import concourse.tile as tile
    from concourse import mybir
    from concourse.masks import make_identity

    F32 = mybir.dt.float32
    F32R = mybir.dt.float32r
    AF = mybir.ActivationFunctionType
    OP = mybir.AluOpType
    AX = mybir.AxisListType

    nc = bacc.Bacc()

    # ---------------- DRAM ----------------
    x_in = nc.dram_tensor("x", [T, PD], F32R, kind="ExternalInput")
    mask2 = nc.dram_tensor("mask2", [P, P], F32, kind="ExternalInput")
    w = {}
    for name in ("w_q1", "w_kv1", "w_o1", "w_q2", "w_kv2", "w_o2"):
        w[name] = nc.dram_tensor(name, [PD, PD], F32R, kind="ExternalInput")
    w["w_l1"] = nc.dram_tensor("w_l1", [PD, H], F32R, kind="ExternalInput")
    w["w_l2"] = nc.dram_tensor("w_l2", [H, PD], F32R, kind="ExternalInput")
    bias = {}
    if flags["use_bias"]:
        for name in ("b_q1", "b_kv1", "b_o1", "b_q2", "b_kv2", "b_o2",
                     "b_l2"):
            bias[name] = nc.dram_tensor(name, [PD], F32R, kind="ExternalInput")
        bias["b_l1"] = nc.dram_tensor("b_l1", [H], F32R, kind="ExternalInput")
    gains = {}
    if flags["use_gains"]:
        for name in ("g1", "be1", "g2", "be2", "g3", "be3"):
            gains[name] = nc.dram_tensor(name, [PD], F32, kind="ExternalInput")
    out = nc.dram_tensor("out", [T, PD], F32, kind="ExternalOutput")

    dbg_kind0 = "ExternalOutput" if flags.get("debug") else "Internal"
    kvt_hbm = nc.dram_tensor("kvt_hbm", [KT, P, T], F32R, kind=dbg_kind0)
    kvo_hbm = nc.dram_tensor("kvo_hbm", [T, PD], F32R, kind=dbg_kind0)
    kv2t_hbm = nc.dram_tensor("kv2t_hbm", [KT, P, NG, V, GP], F32R)
    kvo2_hbm = nc.dram_tensor("kvo2_hbm", [NG, P, PD], F32R)
    dbg_kind = "ExternalOutput" if flags.get("debug") else "Internal"
    x1_hbm = nc.dram_tensor("x1_hbm", [T, PD], F32R, kind=dbg_kind)
    x2_hbm = nc.dram_tensor("x2_hbm", [T, PD], F32R, kind=dbg_kind)
    h1_hbm = nc.dram_tensor("h1_hbm", [HT, P, T], F32R)
    if flags.get("debug"):
        attn_dbg = nc.dram_tensor("attn_dbg", [V, P, P], F32R,
                                  kind="ExternalOutput")
        xr1_dbg = nc.dram_tensor("xr1_dbg", [T, PD], F32,
                                 kind="ExternalOutput")
    else:
        attn_dbg = xr1_dbg = None

    def bcast_row(src_ap, dst_tile):
        bc = bass.AP(tensor=src_ap.tensor, offset=src_ap.offset,
                     ap=[[0, P]] + list(src_ap.ap))
        nc.sync.dma_start(out=dst_tile, in_=bc)

    with tile.TileContext(nc) as tc, ExitStack() as glb:
        const = glb.enter_context(tc.tile_pool(name="const", bufs=1))
        ident_f = const.tile([P, P], F32, name="ident_f")
        make_identity(nc, ident_f)
        ident = const.tile([P, P], F32R, name="ident")
        nc.vector.tensor_copy(ident, ident_f)
        eps_t = const.tile([P, 1], F32, name="eps_t")
        nc.vector.memset(eps_t, EPS)
        mask_sb = const.tile([P, P], F32, name="mask_sb")
        nc.sync.dma_start(out=mask_sb, in_=mask2[:, :])

        bias_sb = {}
        if flags["use_bias"]:
            for name in ("b_q1", "b_kv1", "b_q2", "b_kv2"):
                t_ = const.tile([P, KT], F32, name=name)
                nc.sync.dma_start(
                    out=t_,
                    in_=bias[name][:].rearrange("(m p) -> p m", p=P).bitcast(F32))
                bias_sb[name] = t_
            t_ = const.tile([P, HT], F32, name="b_l1")
            nc.sync.dma_start(
                out=t_,
                in_=bias["b_l1"][:].rearrange("(m p) -> p m", p=P).bitcast(F32))
            bias_sb["b_l1"] = t_
            for name in ("b_o1", "b_o2", "b_l2"):
                t_ = const.tile([P, PD], F32, name=name)
                bcast_row(bias[name][:].bitcast(F32), t_)
                bias_sb[name] = t_
        gain_sb = {}
        if flags["use_gains"]:
            for name in ("g1", "be1", "g2", "be2", "g3", "be3"):
                t_ = const.tile([P, PD], F32, name=name)
                bcast_row(gains[name][:], t_)
                gain_sb[name] = t_

        # big reusable region + attn smalls: closed before P3b
        reg_ctx = ExitStack()
        regp = reg_ctx.enter_context(tc.tile_pool(name="region", bufs=1))
        reg = regp.tile([P, KT, T], F32R, name="reg")   # 16.8 MB

        # late-bound pool refs used by helpers
        pools = {}

        def evict(dst, src_ps):
            nc.vector.tensor_copy(dst, src_ps)

        def ln_norm(tmp, dst, g_t, be_t):
            """dst = LN(tmp); tmp token-major [P, PD] F32."""
            lnp = pools["lnp"]
            stats = lnp.tile([P, NT, 6], F32, name="stats")
            tv = tmp.rearrange("p (n c) -> p n c", n=NT)
            for j in range(NT):
                nc.vector.bn_stats(out=stats[:, j, :], in_=tv[:, j, :])
            mv = lnp.tile([P, 2], F32, name="mv")
            nc.vector.bn_aggr(out=mv, in_=stats)
            std = lnp.tile([P, 1], F32, name="std")
            nc.scalar.activation(out=std, in_=mv[:, 1:2], func=AF.Sqrt,
                                 bias=eps_t, scale=1.0)
            rstd = lnp.tile([P, 1], F32, name="rstd")
            nc.vector.reciprocal(out=rstd, in_=std)
            if g_t is None:
                nc.vector.tensor_scalar(
                    out=dst, in0=tmp, scalar1=mv[:, 0:1], scalar2=rstd,
                    op0=OP.subtract, op1=OP.mult)
            else:
                nrm = lnp.tile([P, PD], F32, name="nrm")
                nc.vector.tensor_scalar(
                    out=nrm, in0=tmp, scalar1=mv[:, 0:1], scalar2=rstd,
                    op0=OP.subtract, op1=OP.mult)
                nc.vector.tensor_mul(nrm, nrm, g_t)
                nc.vector.tensor_add(dst, nrm, be_t)

        def softmax_transpose(sc_src, masked, attnT_dst, recip_dst):
            smp, trp = pools["smp"], pools["trp"]
            if masked:
                sc_sb = smp.tile([P, P], F32, name="sc_sb")
                nc.vector.tensor_add(sc_sb, sc_src, mask_sb)
                src = sc_sb
            else:
                src = sc_src
            mx = smp.tile([P, 1], F32, name="mx")
            nc.vector.tensor_reduce(out=mx, in_=src, axis=AX.X, op=OP.max)
            nbias = smp.tile([P, 1], F32, name="nbias")
            nc.vector.tensor_scalar_mul(nbias, mx, -SCALE)
            aexp = smp.tile([P, P], F32R, name="aexp")
            sums = smp.tile([P, 1], F32, name="sums")
            nc.scalar.activation(out=aexp, in_=src, func=AF.Exp,
                                 bias=nbias, scale=SCALE, accum_out=sums)
            nc.vector.reciprocal(out=recip_dst, in_=sums)
            pt = trp.tile([P, P], F32R, name="pt")
            nc.tensor.transpose(pt, aexp, ident)
            evict(attnT_dst, pt)
            return aexp

        # ================= P0: x -> XT =================
        with tc.tile_pool(name="p0tok", bufs=3) as tokp, \
             tc.tile_pool(name="p0ps", bufs=4, space="PSUM") as trp0:
            for v in range(V):
                xv = tokp.tile([P, PD], F32R, name="xv")
                nc.sync.dma_start(out=xv, in_=x_in[v * P:(v + 1) * P, :])
                for j in range(KT):
                    pt = trp0.tile([P, P], F32R, name="pt")
                    nc.tensor.transpose(pt, xv[:, j * P:(j + 1) * P], ident)
                    evict(reg[:, j, v * P:(v + 1) * P], pt)

        # ============ attention layer (emitted twice) ============
        def attn_layer(lyr, attnp):
            aT = attnp.tile([P, V, P], F32R, name=f"attnT{lyr}")
            rc = attnp.tile([P, V], F32, name=f"recip{lyr}")
            if lyr == 1:
                wq, wkv, wo = w["w_q1"], w["w_kv1"], w["w_o1"]
                kvt_sp, kvo_sp, xres_hbm, xout_hbm = (
                    kvt_hbm, kvo_hbm, x_in, x1_hbm)
                g_pair = (gain_sb.get("g1"), gain_sb.get("be1"))
                bq, bkv, bo = (bias_sb.get("b_q1"), bias_sb.get("b_kv1"),
                               bias_sb.get("b_o1"))
            else:
                wq, wkv, wo = w["w_q2"], w["w_kv2"], w["w_o2"]
                kvt_sp, kvo_sp, xres_hbm, xout_hbm = (
                    kv2t_hbm, kvo2_hbm, x1_hbm, x2_hbm)
                g_pair = (gain_sb.get("g2"), gain_sb.get("be2"))
                bq, bkv, bo = (bias_sb.get("b_q2"), bias_sb.get("b_kv2"),
                               bias_sb.get("b_o2"))

            def lcols(stage_m, i):
                return stage_m[:, i * P:(i + 1) * P]

            def stage_dst(stage_m, n):
                """eviction dst for token chunk n; g-blocked when lyr 2."""
                if lyr == 1:
                    return stage_m[:, n * NCH:(n + 1) * NCH]
                r = stage_m.rearrange("d (g v pl) -> d v g pl", g=NG, v=V)
                return r[:, 4 * n:4 * n + 4, :, :]

            def ps_src(ps, n):
                if lyr == 1:
                    return ps
                return ps.rearrange("d (v g pl) -> d v g pl", v=4, g=NG)

            # ---- a: QT/KVT + inline scores + KVT spill ----
            with tc.tile_pool(name="a_sc", bufs=1, space="PSUM") as scp:
                sc_ps = [scp.tile([P, 4 * P], F32, name=f"sc{i}")
                         for i in range(4)]
                with tc.tile_pool(name="a_w", bufs=2) as wp, \
                     tc.tile_pool(name="a_st", bufs=2) as stp, \
                     tc.tile_pool(name="a_ps", bufs=2, space="PSUM") as pp:
                    for m in range(KT):
                        wq_m = wp.tile([P, KT, P], F32R, name="wq_m")
                        nc.sync.dma_start(
                            out=wq_m,
                            in_=wq[:, m * P:(m + 1) * P].rearrange(
                                "(kt p) c -> p kt c", p=P))
                        wk_m = wp.tile([P, KT, P], F32R, name="wk_m")
                        nc.sync.dma_start(
                            out=wk_m,
                            in_=wkv[:, m * P:(m + 1) * P].rearrange(
                                "(kt p) c -> p kt c", p=P))
                        qt_m = stp.tile([P, T], F32R, name="qt_m")
                        kvt_m = stp.tile([P, T], F32R, name="kvt_m")
                        for n in range(NT):
                            ps_q = pp.tile([P, NCH], F32, name="ps_q")
                            ps_k = pp.tile([P, NCH], F32, name="ps_k")
                            for k in range(KT):
                                nc.tensor.matmul(
                                    ps_q, wq_m[:, k, :],
                                    reg[:, k, n * NCH:(n + 1) * NCH],
                                    start=(k == 0), stop=(k == KT - 1))
                            for k in range(KT):
                                nc.tensor.matmul(
                                    ps_k, wk_m[:, k, :],
                                    reg[:, k, n * NCH:(n + 1) * NCH],
                                    start=(k == 0), stop=(k == KT - 1))
                            if bq is not None:
                                nc.scalar.activation(
                                    out=stage_dst(qt_m, n), in_=ps_src(ps_q, n),
                                    func=AF.Identity,
                                    bias=bq[:, m:m + 1], scale=1.0)
                                nc.scalar.activation(
                                    out=stage_dst(kvt_m, n), in_=ps_src(ps_k, n),
                                    func=AF.Identity,
                                    bias=bkv[:, m:m + 1], scale=1.0)
                            else:
                                evict(stage_dst(qt_m, n), ps_src(ps_q, n))
                                evict(stage_dst(kvt_m, n), ps_src(ps_k, n))
                        if lyr == 1:
                            nc.gpsimd.dma_start(out=kvt_sp[m], in_=kvt_m)
                        else:
                            nc.gpsimd.dma_start(
                                out=kvt_sp[m].rearrange("p g v pl -> p (g v pl)"),
                                in_=kvt_m)
                        for i in range(V):
                            # start clears the whole PSUM bank: only the
                            # first matmul touching each bank may set it.
                            nc.tensor.matmul(
                                sc_ps[i // 4][:, (i % 4) * P:(i % 4 + 1) * P],
                                lcols(qt_m, i), lcols(kvt_m, i),
                                start=(m == 0 and i % 4 == 0),
                                stop=(m == KT - 1), skip_group_check=True)

                # ---- c: softmax + attn transpose ----
                with tc.tile_pool(name="c_sm", bufs=3) as smp_, \
                     tc.tile_pool(name="c_tr", bufs=2, space="PSUM") as trp_:
                    pools["smp"], pools["trp"] = smp_, trp_
                    for i in range(V):
                        aexp_i = softmax_transpose(
                            sc_ps[i // 4][:, (i % 4) * P:(i % 4 + 1) * P],
                            lyr == 2, aT[:, i, :], rc[:, i:i + 1])
                        if attn_dbg is not None and lyr == 1:
                            nc.gpsimd.dma_start(out=attn_dbg[i], in_=aexp_i)

            # ---- KVT back, resident in region ----
            if lyr == 1:
                nc.sync.dma_start(
                    out=reg, in_=kvt_sp[:, :, :].rearrange("m p t -> p m t"))
            else:
                nc.sync.dma_start(
                    out=reg,
                    in_=kv2t_hbm[:, :, :, :, :].rearrange(
                        "m p g v pl -> p m (g v pl)"))

            # ---- b: KVo = KV @ Wo (token-major), spill ----
            with tc.tile_pool(name="b_w", bufs=3) as wop, \
                 tc.tile_pool(name="b_st", bufs=3) as bst, \
                 tc.tile_pool(name="b_ps", bufs=1, space="PSUM") as bpp:
                for half in range(2):
                    for f in range(NT):
                        pss = [bpp.tile([P, NCH], F32, name=f"bps{i}")
                               for i in range(8)]
                        for k in range(KT):
                            wo_kf = wop.tile([P, NCH], F32R, name="wo_kf")
                            nc.sync.dma_start(
                                out=wo_kf,
                                in_=wo[k * P:(k + 1) * P,
                                       f * NCH:(f + 1) * NCH])
                            for t8 in range(8):
                                ti = half * 8 + t8
                                nc.tensor.matmul(
                                    pss[t8],
                                    reg[:, k, ti * P:(ti + 1) * P],
                                    wo_kf,
                                    start=(k == 0), stop=(k == KT - 1))
                        for t8 in range(8):
                            ti = half * 8 + t8
                            st = bst.tile([P, NCH], F32R, name="bstg")
                            evict(st, pss[t8])
                            if lyr == 1:
                                dst = kvo_sp[ti * P:(ti + 1) * P,
                                             f * NCH:(f + 1) * NCH]
                            else:
                                dst = kvo_sp[ti, :, f * NCH:(f + 1) * NCH]
                            nc.gpsimd.dma_start(out=dst, in_=st)

            # ---- d: a = attnT^T @ KVo; residual + LN; next XT ----
            NB = V if lyr == 1 else NG
            with tc.tile_pool(name="d_io", bufs=2) as iop, \
                 tc.tile_pool(name="d_tmp", bufs=2) as tmpp, \
                 tc.tile_pool(name="d_ln", bufs=4) as lnp_, \
                 tc.tile_pool(name="d_ps", bufs=4, space="PSUM") as app, \
                 tc.tile_pool(name="d_tr", bufs=2, space="PSUM") as trp_:
                pools["lnp"] = lnp_
                for i in range(NB):
                    kvo_i = iop.tile([P, PD], F32R, name="kvo_i")
                    if lyr == 1:
                        nc.sync.dma_start(
                            out=kvo_i, in_=kvo_sp[i * P:(i + 1) * P, :])
                    else:
                        nc.sync.dma_start(out=kvo_i, in_=kvo_sp[i])
                    xr = iop.tile([P, PD], F32, name="xr")
                    if lyr == 1:
                        nc.sync.dma_start(
                            out=xr,
                            in_=xres_hbm[i * P:(i + 1) * P, :].bitcast(F32))
                    else:
                        for v in range(V):
                            r0 = v * P + i * GP
                            nc.sync.dma_start(
                                out=xr[v * GP:(v + 1) * GP, :],
                                in_=xres_hbm[r0:r0 + GP, :].bitcast(F32))
                    for f in range(NT):
                        ps_a = app.tile([P, NCH], F32, name="ps_a")
                        nc.tensor.matmul(
                            ps_a, aT[:, i, :],
                            kvo_i[:, f * NCH:(f + 1) * NCH],
                            start=True, stop=True)
                        sl = slice(f * NCH, (f + 1) * NCH)
                        nc.vector.scalar_tensor_tensor(
                            out=xr[:, sl], in0=ps_a, scalar=rc[:, i:i + 1],
                            in1=xr[:, sl], op0=OP.mult, op1=OP.add)
                    if bo is not None:
                        nc.vector.tensor_add(xr, xr, bo)
                    if xr1_dbg is not None and lyr == 1:
                        nc.gpsimd.dma_start(
                            out=xr1_dbg[i * P:(i + 1) * P, :], in_=xr)
                    xo = tmpp.tile([P, PD], F32R, name="xo")
                    ln_norm(xr, xo, g_pair[0], g_pair[1])
                    if lyr == 1:
                        nc.gpsimd.dma_start(
                            out=xout_hbm[i * P:(i + 1) * P, :], in_=xo)
                    else:
                        for v in range(V):
                            r0 = v * P + i * GP
                            nc.gpsimd.dma_start(
                                out=xout_hbm[r0:r0 + GP, :],
                                in_=xo[v * GP:(v + 1) * GP, :])
                    for j in range(KT):
                        pt = trp_.tile([P, P], F32R, name="pt")
                        nc.tensor.transpose(
                            pt, xo[:, j * P:(j + 1) * P], ident)
                        if lyr == 1:
                            evict(reg[:, j, i * P:(i + 1) * P], pt)
                        else:
                            rr = reg.rearrange(
                                "d m (v g pl) -> d m v g pl", v=V, g=NG)
                            evict(rr[:, j, :, i, :],
                                  pt.rearrange("d (v pl) -> d v pl", v=V))

        for _lyr in (1, 2):
            with tc.tile_pool(name=f"attnp{_lyr}", bufs=1) as _ap:
                attn_layer(_lyr, _ap)

        # ================= P3a: h1 = gelu(W_l1^T X2) =================
        bl1 = bias_sb.get("b_l1")
        with tc.tile_pool(name="f_w", bufs=2) as wp, \
             tc.tile_pool(name="f_st", bufs=2) as stp, \
             tc.tile_pool(name="f_ps", bufs=4, space="PSUM") as pp:
            for m in range(HT):
                wl_m = wp.tile([P, KT, P], F32R, name="wl_m")
                nc.sync.dma_start(
                    out=wl_m,
                    in_=w["w_l1"][:, m * P:(m + 1) * P].rearrange(
                        "(kt p) c -> p kt c", p=P))
                h1st = stp.tile([P, T], F32R, name="h1st")
                for n in range(NT):
                    ps_h = pp.tile([P, NCH], F32, name="ps_h")
                    for k in range(KT):
                        nc.tensor.matmul(
                            ps_h, wl_m[:, k, :],
                            reg[:, k, n * NCH:(n + 1) * NCH],
                            start=(k == 0), stop=(k == KT - 1))
                    nc.scalar.activation(
                        out=h1st[:, n * NCH:(n + 1) * NCH], in_=ps_h,
                        func=AF.Gelu,
                        bias=(bl1[:, m:m + 1] if bl1 is not None else 0.0),
                        scale=1.0)
                nc.gpsimd.dma_start(out=h1_hbm[m], in_=h1st)

        # region + attn tiles die here
        reg_ctx.close()

        # ============ P3b/c: X3 = X2 + h1^T Wl2; LN3 -> out ============
        KG = 4
        g3_pair = (gain_sb.get("g3"), gain_sb.get("be3"))
        bl2 = bias_sb.get("b_l2")
        with tc.tile_pool(name="x3p", bufs=1) as x3p, \
             tc.tile_pool(name="h_in", bufs=2) as hp, \
             tc.tile_pool(name="w2_in", bufs=2) as w2p, \
             tc.tile_pool(name="x3ps", bufs=4, space="PSUM") as pp3, \
             tc.tile_pool(name="x3ln", bufs=4) as lnp3, \
             tc.tile_pool(name="x3out", bufs=1) as outp:
            pools["lnp"] = lnp3
            for halfb in range(2):
                x3 = x3p.tile([P, 8, PD], F32, name="x3")
                for t8 in range(8):
                    ti = halfb * 8 + t8
                    nc.sync.dma_start(
                        out=x3[:, t8, :],
                        in_=x2_hbm[ti * P:(ti + 1) * P, :].bitcast(F32))
                    if bl2 is not None:
                        nc.vector.tensor_add(x3[:, t8, :], x3[:, t8, :], bl2)
                for kg in range(HT // KG):
                    h1g = hp.tile([P, KG, T], F32R, name="h1g")
                    nc.sync.dma_start(
                        out=h1g,
                        in_=h1_hbm[kg * KG:(kg + 1) * KG].rearrange(
                            "m p t -> p m t"))
                    w2g = w2p.tile([P, KG, PD], F32R, name="w2g")
                    nc.sync.dma_start(
                        out=w2g,
                        in_=w["w_l2"][kg * KG * P:(kg + 1) * KG * P, :]
                        .rearrange("(m p) c -> p m c", p=P))
                    for t8 in range(8):
                        ti = halfb * 8 + t8
                        for f in range(NT):
                            ps3 = pp3.tile([P, NCH], F32, name="ps3")
                            for k in range(KG):
                                nc.tensor.matmul(
                                    ps3, h1g[:, k, ti * P:(ti + 1) * P],
                                    w2g[:, k, f * NCH:(f + 1) * NCH],
                                    start=(k == 0), stop=(k == KG - 1))
                            sl = slice(f * NCH, (f + 1) * NCH)
                            nc.vector.tensor_add(
                                x3[:, t8, sl], x3[:, t8, sl], ps3)
                for t8 in range(8):
                    ti = halfb * 8 + t8
                    ot = outp.tile([P, PD], F32, name="ot")
                    ln_norm(x3[:, t8, :], ot, g3_pair[0], g3_pair[1])
                    nc.gpsimd.dma_start(
                        out=out[ti * P:(ti + 1) * P, :], in_=ot)

    return nc


# ======================= host side =======================

_CACHE = {}


def _get_runner(flags):
    flags_key = (flags["use_bias"], flags["use_gains"], flags.get("debug", False))
    if flags_key in _CACHE:
        return _CACHE[flags_key]
    import jax
    from jax.experimental.shard_map import shard_map
    from jax.sharding import Mesh, NamedSharding, PartitionSpec
    import concourse.mybir as mybir
    from concourse import bass2jax

    nc = build_encoder(flags)
    bass2jax.install_neuronx_cc_hook()
    if not nc.is_finalized():
        nc.finalize()

    partition_name = (
        nc.partition_id_tensor.name if nc.partition_id_tensor else None)
    in_names, out_names, out_avals, zero_outs = [], [], [], []
    for alloc in nc.m.functions[0].allocations:
        if not isinstance(alloc, mybir.MemoryLocationSet):
            continue
        name = alloc.memorylocations[0].name
        if alloc.kind == "ExternalInput":
            if name != partition_name:
                in_names.append(name)
        elif alloc.kind == "ExternalOutput":
            shape = tuple(alloc.tensor_shape)
            dtype = mybir.dt.np(alloc.dtype)
            out_names.append(name)
            out_avals.append(jax.core.ShapedArray(shape, dtype))
            zero_outs.append(np.zeros(shape, dtype))
    all_in_names = list(in_names) + list(out_names)
    if partition_name is not None:
        all_in_names.append(partition_name)

    def _body(*args):
        operands = list(args)
        if partition_name is not None:
            operands.append(bass2jax.partition_id_tensor())
        outs = bass2jax._bass_exec_p.bind(
            *operands,
            out_avals=tuple(out_avals),
            in_names=tuple(all_in_names),
            out_names=tuple(out_names),
            lowering_input_output_aliases=(),
            sim_require_finite=False,
            sim_require_nnan=False,
            nc=nc)
        return tuple(outs)

    devices = jax.devices()[:B]
    mesh = Mesh(np.asarray(devices), ("core",))
    spec = PartitionSpec("core")
    sharding = NamedSharding(mesh, spec)
    n_io = len(in_names) + len(out_names)
    fn = jax.jit(
        shard_map(_body, mesh=mesh, in_specs=(spec,) * n_io,
                  out_specs=(spec,) * len(out_names), check_rep=False),
        keep_unused=True)
    r = {"fn": fn, "in_names": in_names, "out_names": out_names,
         "zero_outs": zero_outs, "sharding": sharding}
    _CACHE[flags_key] = r
    return r


def prepare_inputs(inputs):
    """Full inputs -> (flags, per-core input maps)."""
    x = np.ascontiguousarray(
        np.asarray(inputs["x"], dtype=np.float32).reshape(B, T, PD))
    var_mask = np.asarray(inputs["var_mask"])

    def z(name):
        return not np.any(np.asarray(inputs[name]))

    use_bias = not all(z(n) for n in (
        "b_q1", "b_kv1", "b_o1", "b_q2", "b_kv2", "b_o2", "b_l1", "b_l2"))
    g_triv = (np.all(np.asarray(inputs["g1"]) == 1.0)
              and np.all(np.asarray(inputs["g2"]) == 1.0)
              and np.all(np.asarray(inputs["g3"]) == 1.0)
              and z("be1") and z("be2") and z("be3"))
    flags = {"use_bias": use_bias, "use_gains": not g_triv}

    pl_neq = (np.arange(GP)[:, None] != np.arange(GP)[None, :])
    weights = {}
    for n in ("w_q1", "w_kv1", "w_o1", "w_q2", "w_kv2", "w_o2",
              "w_l1", "w_l2"):
        weights[n] = np.ascontiguousarray(np.asarray(inputs[n], np.float32))
    in_maps = []
    for b in range(B):
        m4 = np.where(
            var_mask[b][:, None, :, None] | pl_neq[None, :, None, :],
            np.float32(NEG), np.float32(0.0))            # [v, pl, r, pl2]
        im = {"x": x[b],
              "mask2": np.ascontiguousarray(m4.reshape(P, P))}
        im.update(weights)
        if use_bias:
            for n in ("b_q1", "b_kv1", "b_o1", "b_q2", "b_kv2", "b_o2",
                      "b_l1", "b_l2"):
                im[n] = np.ascontiguousarray(np.asarray(inputs[n], np.float32))
        if flags["use_gains"]:
            for n in ("g1", "be1", "g2", "be2", "g3", "be3"):
                im[n] = np.ascontiguousarray(np.asarray(inputs[n], np.float32))
        in_maps.append(im)
    return flags, in_maps


def run_on_device(flags, in_maps, time_iters=0):
    import jax
    r = _get_runner(flags)
    concat = [np.concatenate([m[n] for m in in_maps], axis=0)
              for n in r["in_names"]]
    concat += [np.zeros((B * z.shape[0], *z.shape[1:]), z.dtype)
               for z in r["zero_outs"]]
    dev_in = [jax.device_put(a, r["sharding"]) for a in concat]
    jax.block_until_ready(dev_in)
    outs = r["fn"](*dev_in)
    jax.block_until_ready(outs)
    burst = None
    if time_iters:
        t0 = time.perf_counter()
        for _ in range(time_iters):
            outs = r["fn"](*dev_in)
        jax.block_until_ready(outs)
        burst = (time.perf_counter() - t0) / time_iters
    out_full = np.asarray(outs[r["out_names"].index("out")])
    return out_full.reshape(B, T, PD), burst


def kernel(**inputs):
    flags, in_maps = prepare_inputs(inputs)
    out, _ = run_on_device(flags, in_maps)
    x_out = out.reshape(B, V, P, PD).astype(np.float32)
    x_next = x_out.reshape(B, V, P // 2, 2 * L, D)
    return x_out, x_next


# revision 12
# speedup vs baseline: 3.9231x; 3.9231x over previous
"""Trainium2 Bass kernel for nn_Encoder_Cross (sparse_attention).

Per batch element b (8 of them, one per NeuronCore):
  x [V=16, P=128, PD=2048]; attn1 per-variable over patches; attn2
  per-patch over variables (masked); FFN 2048->8192->2048 (exact gelu);
  residuals + 3 LayerNorms. Outputs (x_out, x_next=reshape(x_out)).

Sharding: pure data-parallel over B (B=8 == 8 cores), no collectives.

All matmuls run as float32r (full PE rate, ~tf32 rounding). One 16.8MB
SBUF region is reused through the phases: XT -> KVT -> X1T -> KV2T -> X2T;
large intermediates (KVT, KVo, h1, X1, X2) spill through HBM.
"""

import math
import os
import time
from contextlib import ExitStack

import numpy as np

os.environ.setdefault("JAX_PLATFORMS", "axon,cpu")

B, V, P, L, D = 8, 16, 128, 16, 128
PD = L * D            # 2048
T = V * P             # 2048 tokens per core
KT = PD // P          # 16 k tiles
NCH = 512             # moving chunk
NT = T // NCH         # 4 token chunks
H = 4 * PD            # 8192
HT = H // P           # 64 h tiles
GP = P // V           # 8 patches per group
NG = P // GP          # 16 groups
SCALE = 1.0 / math.sqrt(PD)
EPS = 1e-5
NEG = -1.0e30


def build_encoder(flags):
    import concourse.bass as bass
    import concourse.bacc as bacc
    import concourse.tile as tile
    from concourse import mybir
    from concourse.masks import make_identity

    F32 = mybir.dt.float32
    F32R = mybir.dt.float32r
    AF = mybir.ActivationFunctionType
    OP = mybir.AluOpType
    AX = mybir.AxisListType

    nc = bacc.Bacc()

    # ---------------- DRAM ----------------
    x_in = nc.dram_tensor("x", [T, PD], F32R, kind="ExternalInput")
    mask2 = nc.dram_tensor("mask2", [P, P], F32, kind="ExternalInput")
    w = {}
    for name in ("w_q1", "w_kv1", "w_o1", "w_q2", "w_kv2", "w_o2"):
        w[name] = nc.dram_tensor(name, [PD, PD], F32R, kind="ExternalInput")
    w["w_l1"] = nc.dram_tensor("w_l1", [PD, H], F32R, kind="ExternalInput")
    w["w_l2"] = nc.dram_tensor("w_l2", [H, PD], F32R, kind="ExternalInput")
    bias = {}
    if flags["use_bias"]:
        for name in ("b_q1", "b_kv1", "b_o1", "b_q2", "b_kv2", "b_o2",
                     "b_l2"):
            bias[name] = nc.dram_tensor(name, [PD], F32R, kind="ExternalInput")
        bias["b_l1"] = nc.dram_tensor("b_l1", [H], F32R, kind="ExternalInput")
    gains = {}
    if flags["use_gains"]:
        for name in ("g1", "be1", "g2", "be2", "g3", "be3"):
            gains[name] = nc.dram_tensor(name, [PD], F32, kind="ExternalInput")
    out = nc.dram_tensor("out", [T, PD], F32, kind="ExternalOutput")

    dbg_kind0 = "ExternalOutput" if flags.get("debug") else "Internal"
    kvt_hbm = nc.dram_tensor("kvt_hbm", [KT, P, T], F32R, kind=dbg_kind0)
    kvo_hbm = nc.dram_tensor("kvo_hbm", [T, PD], F32R, kind=dbg_kind0)
    kv2t_hbm = nc.dram_tensor("kv2t_hbm", [KT, P, NG, V, GP], F32R)
    kvo2_hbm = nc.dram_tensor("kvo2_hbm", [NG, P, PD], F32R)
    dbg_kind = "ExternalOutput" if flags.get("debug") else "Internal"
    x1_hbm = nc.dram_tensor("x1_hbm", [T, PD], F32R, kind=dbg_kind)
    x2_hbm = nc.dram_tensor("x2_hbm", [T, PD], F32R, kind=dbg_kind)
    h1_hbm = nc.dram_tensor("h1_hbm", [HT, P, T], F32R)
    if flags.get("debug"):
        attn_dbg = nc.dram_tensor("attn_dbg", [V, P, P], F32R,
                                  kind="ExternalOutput")
        xr1_dbg = nc.dram_tensor("xr1_dbg", [T, PD], F32,
                                 kind="ExternalOutput")
    else:
        attn_dbg = xr1_dbg = None

    def bcast_row(src_ap, dst_tile):
        bc = bass.AP(tensor=src_ap.tensor, offset=src_ap.offset,
                     ap=[[0, P]] + list(src_ap.ap))
        nc.sync.dma_start(out=dst_tile, in_=bc)

    with tile.TileContext(nc) as tc, ExitStack() as glb:
        const = glb.enter_context(tc.tile_pool(name="const", bufs=1))
        ident_f = const.tile([P, P], F32, name="ident_f")
        make_identity(nc, ident_f)
        ident = const.tile([P, P], F32R, name="ident")
        nc.vector.tensor_copy(ident, ident_f)
        eps_t = const.tile([P, 1], F32, name="eps_t")
        nc.vector.memset(eps_t, EPS)
        mask_sb = const.tile([P, P], F32, name="mask_sb")
        nc.sync.dma_start(out=mask_sb, in_=mask2[:, :])

        bias_sb = {}
        if flags["use_bias"]:
            for name in ("b_q1", "b_kv1", "b_q2", "b_kv2"):
                t_ = const.tile([P, KT], F32, name=name)
                nc.sync.dma_start(
                    out=t_,
                    in_=bias[name][:].rearrange("(m p) -> p m", p=P).bitcast(F32))
                bias_sb[name] = t_
            t_ = const.tile([P, HT], F32, name="b_l1")
            nc.sync.dma_start(
                out=t_,
                in_=bias["b_l1"][:].rearrange("(m p) -> p m", p=P).bitcast(F32))
            bias_sb["b_l1"] = t_
            for name in ("b_o1", "b_o2", "b_l2"):
                t_ = const.tile([P, PD], F32, name=name)
                bcast_row(bias[name][:].bitcast(F32), t_)
                bias_sb[name] = t_
        gain_sb = {}
        if flags["use_gains"]:
            for name in ("g1", "be1", "g2", "be2", "g3", "be3"):
                t_ = const.tile([P, PD], F32, name=name)
                bcast_row(gains[name][:], t_)
                gain_sb[name] = t_

        # big reusable region + attn smalls: closed before P3b
        reg_ctx = ExitStack()
        regp = reg_ctx.enter_context(tc.tile_pool(name="region", bufs=1))
        reg = regp.tile([P, KT, T], F32R, name="reg")   # 16.8 MB

        # late-bound pool refs used by helpers
        pools = {}

        def evict(dst, src_ps):
            nc.vector.tensor_copy(dst, src_ps)

        def ln_norm(tmp, dst, g_t, be_t):
            """dst = LN(tmp); tmp token-major [P, PD] F32."""
            lnp = pools["lnp"]
            stats = lnp.tile([P, NT, 6], F32, name="stats")
            tv = tmp.rearrange("p (n c) -> p n c", n=NT)
            for j in range(NT):
                nc.vector.bn_stats(out=stats[:, j, :], in_=tv[:, j, :])
            mv = lnp.tile([P, 2], F32, name="mv")
            nc.vector.bn_aggr(out=mv, in_=stats)
            std = lnp.tile([P, 1], F32, name="std")
            nc.scalar.activation(out=std, in_=mv[:, 1:2], func=AF.Sqrt,
                                 bias=eps_t, scale=1.0)
            rstd = lnp.tile([P, 1], F32, name="rstd")
            nc.vector.reciprocal(out=rstd, in_=std)
            if g_t is None:
                nc.vector.tensor_scalar(
                    out=dst, in0=tmp, scalar1=mv[:, 0:1], scalar2=rstd,
                    op0=OP.subtract, op1=OP.mult)
            else:
                nrm = lnp.tile([P, PD], F32, name="nrm")
                nc.vector.tensor_scalar(
                    out=nrm, in0=tmp, scalar1=mv[:, 0:1], scalar2=rstd,
                    op0=OP.subtract, op1=OP.mult)
                nc.vector.tensor_mul(nrm, nrm, g_t)
                nc.vector.tensor_add(dst, nrm, be_t)

        def softmax_transpose(sc_src, masked, attnT_dst, recip_dst):
            smp, trp = pools["smp"], pools["trp"]
            if masked:
                sc_sb = smp.tile([P, P], F32, name="sc_sb")
                nc.vector.tensor_add(sc_sb, sc_src, mask_sb)
                src = sc_sb
            else:
                src = sc_src
            mx = smp.tile([P, 1], F32, name="mx")
            nc.vector.tensor_reduce(out=mx, in_=src, axis=AX.X, op=OP.max)
            nbias = smp.tile([P, 1], F32, name="nbias")
            nc.vector.tensor_scalar_mul(nbias, mx, -SCALE)
            aexp = smp.tile([P, P], F32R, name="aexp")
            sums = smp.tile([P, 1], F32, name="sums")
            nc.scalar.activation(out=aexp, in_=src, func=AF.Exp,
                                 bias=nbias, scale=SCALE, accum_out=sums)
            nc.vector.reciprocal(out=recip_dst, in_=sums)
            pt = trp.tile([P, P], F32R, name="pt")
            nc.tensor.transpose(pt, aexp, ident)
            evict(attnT_dst, pt)
            return aexp

        # ================= P0: x -> XT =================
        with tc.tile_pool(name="p0tok", bufs=3) as tokp, \
             tc.tile_pool(name="p0ps", bufs=4, space="PSUM") as trp0:
            for v in range(V):
                xv = tokp.tile([P, PD], F32R, name="xv")
                nc.sync.dma_start(out=xv, in_=x_in[v * P:(v + 1) * P, :])
                for j in range(KT):
                    pt = trp0.tile([P, P], F32R, name="pt")
                    nc.tensor.transpose(pt, xv[:, j * P:(j + 1) * P], ident)
                    evict(reg[:, j, v * P:(v + 1) * P], pt)

        # ============ attention layer (emitted twice) ============
        def attn_layer(lyr, attnp, upto="full"):
            aT = attnp.tile([P, V, P], F32R, name=f"attnT{lyr}")
            rc = attnp.tile([P, V], F32, name=f"recip{lyr}")
            if lyr == 1:
                wq, wkv, wo = w["w_q1"], w["w_kv1"], w["w_o1"]
                kvt_sp, kvo_sp, xres_hbm, xout_hbm = (
                    kvt_hbm, kvo_hbm, x_in, x1_hbm)
                g_pair = (gain_sb.get("g1"), gain_sb.get("be1"))
                bq, bkv, bo = (bias_sb.get("b_q1"), bias_sb.get("b_kv1"),
                               bias_sb.get("b_o1"))
            else:
                wq, wkv, wo = w["w_q2"], w["w_kv2"], w["w_o2"]
                kvt_sp, kvo_sp, xres_hbm, xout_hbm = (
                    kv2t_hbm, kvo2_hbm, x1_hbm, x2_hbm)
                g_pair = (gain_sb.get("g2"), gain_sb.get("be2"))
                bq, bkv, bo = (bias_sb.get("b_q2"), bias_sb.get("b_kv2"),
                               bias_sb.get("b_o2"))

            def lcols(stage_m, i):
                return stage_m[:, i * P:(i + 1) * P]

            def stage_dst(stage_m, n):
                """eviction dst for token chunk n; g-blocked when lyr 2."""
                if lyr == 1:
                    return stage_m[:, n * NCH:(n + 1) * NCH]
                r = stage_m.rearrange("d (g v pl) -> d v g pl", g=NG, v=V)
                return r[:, 4 * n:4 * n + 4, :, :]

            def ps_src(ps, n):
                if lyr == 1:
                    return ps
                return ps.rearrange("d (v g pl) -> d v g pl", v=4, g=NG)

            # ---- a: QT/KVT + inline scores + KVT spill ----
            with tc.tile_pool(name="a_sc", bufs=1, space="PSUM") as scp:
                sc_ps = [scp.tile([P, 4 * P], F32, name=f"sc{i}")
                         for i in range(4)]
                with tc.tile_pool(name="a_w", bufs=2) as wp, \
                     tc.tile_pool(name="a_st", bufs=2) as stp, \
                     tc.tile_pool(name="a_ps", bufs=2, space="PSUM") as pp:
                    for m in range(KT):
                        wq_m = wp.tile([P, KT, P], F32R, name="wq_m")
                        nc.sync.dma_start(
                            out=wq_m,
                            in_=wq[:, m * P:(m + 1) * P].rearrange(
                                "(kt p) c -> p kt c", p=P))
                        wk_m = wp.tile([P, KT, P], F32R, name="wk_m")
                        nc.sync.dma_start(
                            out=wk_m,
                            in_=wkv[:, m * P:(m + 1) * P].rearrange(
                                "(kt p) c -> p kt c", p=P))
                        qt_m = stp.tile([P, T], F32R, name="qt_m")
                        kvt_m = stp.tile([P, T], F32R, name="kvt_m")
                        for n in range(NT):
                            ps_q = pp.tile([P, NCH], F32, name="ps_q")
                            ps_k = pp.tile([P, NCH], F32, name="ps_k")
                            for k in range(KT):
                                nc.tensor.matmul(
                                    ps_q, wq_m[:, k, :],
                                    reg[:, k, n * NCH:(n + 1) * NCH],
                                    start=(k == 0), stop=(k == KT - 1))
                            for k in range(KT):
                                nc.tensor.matmul(
                                    ps_k, wk_m[:, k, :],
                                    reg[:, k, n * NCH:(n + 1) * NCH],
                                    start=(k == 0), stop=(k == KT - 1))
                            if bq is not None:
                                nc.scalar.activation(
                                    out=stage_dst(qt_m, n), in_=ps_src(ps_q, n),
                                    func=AF.Identity,
                                    bias=bq[:, m:m + 1], scale=1.0)
                                nc.scalar.activation(
                                    out=stage_dst(kvt_m, n), in_=ps_src(ps_k, n),
                                    func=AF.Identity,
                                    bias=bkv[:, m:m + 1], scale=1.0)
                            else:
                                evict(stage_dst(qt_m, n), ps_src(ps_q, n))
                                evict(stage_dst(kvt_m, n), ps_src(ps_k, n))
                        if lyr == 1:
                            nc.gpsimd.dma_start(out=kvt_sp[m], in_=kvt_m)
                        else:
                            nc.gpsimd.dma_start(
                                out=kvt_sp[m].rearrange("p g v pl -> p (g v pl)"),
                                in_=kvt_m)
                        for i in range(V):
                            # start clears the whole PSUM bank: only the
                            # first matmul touching each bank may set it.
                            nc.tensor.matmul(
                                sc_ps[i // 4][:, (i % 4) * P:(i % 4 + 1) * P],
                                lcols(qt_m, i), lcols(kvt_m, i),
                                start=(m == 0 and i % 4 == 0),
                                stop=(m == KT - 1), skip_group_check=True)

                # ---- c: softmax + attn transpose ----
                with tc.tile_pool(name="c_sm", bufs=3) as smp_, \
                     tc.tile_pool(name="c_tr", bufs=2, space="PSUM") as trp_:
                    pools["smp"], pools["trp"] = smp_, trp_
                    for i in range(V):
                        aexp_i = softmax_transpose(
                            sc_ps[i // 4][:, (i % 4) * P:(i % 4 + 1) * P],
                            lyr == 2, aT[:, i, :], rc[:, i:i + 1])
                        if attn_dbg is not None and lyr == 1:
                            nc.gpsimd.dma_start(out=attn_dbg[i], in_=aexp_i)

            if upto == "a":
                return
            # ---- KVT back, resident in region ----
            if lyr == 1:
                nc.sync.dma_start(
                    out=reg, in_=kvt_sp[:, :, :].rearrange("m p t -> p m t"))
            else:
                nc.sync.dma_start(
                    out=reg,
                    in_=kv2t_hbm[:, :, :, :, :].rearrange(
                        "m p g v pl -> p m (g v pl)"))

            # ---- b: KVo = KV @ Wo (token-major), spill ----
            with tc.tile_pool(name="b_w", bufs=3) as wop, \
                 tc.tile_pool(name="b_st", bufs=3) as bst, \
                 tc.tile_pool(name="b_ps", bufs=1, space="PSUM") as bpp:
                for half in range(2):
                    for f in range(NT):
                        pss = [bpp.tile([P, NCH], F32, name=f"bps{i}")
                               for i in range(8)]
                        for k in range(KT):
                            wo_kf = wop.tile([P, NCH], F32R, name="wo_kf")
                            nc.sync.dma_start(
                                out=wo_kf,
                                in_=wo[k * P:(k + 1) * P,
                                       f * NCH:(f + 1) * NCH])
                            for t8 in range(8):
                                ti = half * 8 + t8
                                nc.tensor.matmul(
                                    pss[t8],
                                    reg[:, k, ti * P:(ti + 1) * P],
                                    wo_kf,
                                    start=(k == 0), stop=(k == KT - 1))
                        for t8 in range(8):
                            ti = half * 8 + t8
                            st = bst.tile([P, NCH], F32R, name="bstg")
                            evict(st, pss[t8])
                            if lyr == 1:
                                dst = kvo_sp[ti * P:(ti + 1) * P,
                                             f * NCH:(f + 1) * NCH]
                            else:
                                dst = kvo_sp[ti, :, f * NCH:(f + 1) * NCH]
                            nc.gpsimd.dma_start(out=dst, in_=st)

            # ---- d: a = attnT^T @ KVo; residual + LN; next XT ----
            NB = V if lyr == 1 else NG
            with tc.tile_pool(name="d_io", bufs=2) as iop, \
                 tc.tile_pool(name="d_tmp", bufs=2) as tmpp, \
                 tc.tile_pool(name="d_ln", bufs=4) as lnp_, \
                 tc.tile_pool(name="d_ps", bufs=4, space="PSUM") as app, \
                 tc.tile_pool(name="d_tr", bufs=2, space="PSUM") as trp_:
                pools["lnp"] = lnp_
                for i in range(NB):
                    kvo_i = iop.tile([P, PD], F32R, name="kvo_i")
                    if lyr == 1:
                        nc.sync.dma_start(
                            out=kvo_i, in_=kvo_sp[i * P:(i + 1) * P, :])
                    else:
                        nc.sync.dma_start(out=kvo_i, in_=kvo_sp[i])
                    xr = iop.tile([P, PD], F32, name="xr")
                    if lyr == 1:
                        nc.sync.dma_start(
                            out=xr,
                            in_=xres_hbm[i * P:(i + 1) * P, :].bitcast(F32))
                    else:
                        for v in range(V):
                            r0 = v * P + i * GP
                            nc.sync.dma_start(
                                out=xr[v * GP:(v + 1) * GP, :],
                                in_=xres_hbm[r0:r0 + GP, :].bitcast(F32))
                    for f in range(NT):
                        ps_a = app.tile([P, NCH], F32, name="ps_a")
                        nc.tensor.matmul(
                            ps_a, aT[:, i, :],
                            kvo_i[:, f * NCH:(f + 1) * NCH],
                            start=True, stop=True)
                        sl = slice(f * NCH, (f + 1) * NCH)
                        nc.vector.scalar_tensor_tensor(
                            out=xr[:, sl], in0=ps_a, scalar=rc[:, i:i + 1],
                            in1=xr[:, sl], op0=OP.mult, op1=OP.add)
                    if bo is not None:
                        nc.vector.tensor_add(xr, xr, bo)
                    if xr1_dbg is not None and lyr == 1:
                        nc.gpsimd.dma_start(
                            out=xr1_dbg[i * P:(i + 1) * P, :], in_=xr)
                    xo = tmpp.tile([P, PD], F32R, name="xo")
                    ln_norm(xr, xo, g_pair[0], g_pair[1])
                    if lyr == 1:
                        nc.gpsimd.dma_start(
                            out=xout_hbm[i * P:(i + 1) * P, :], in_=xo)
                    else:
                        for v in range(V):
                            r0 = v * P + i * GP
                            nc.gpsimd.dma_start(
                                out=xout_hbm[r0:r0 + GP, :],
                                in_=xo[v * GP:(v + 1) * GP, :])
                    for j in range(KT):
                        pt = trp_.tile([P, P], F32R, name="pt")
                        nc.tensor.transpose(
                            pt, xo[:, j * P:(j + 1) * P], ident)
                        if lyr == 1:
                            evict(reg[:, j, i * P:(i + 1) * P], pt)
                        else:
                            rr = reg.rearrange(
                                "d m (v g pl) -> d m v g pl", v=V, g=NG)
                            evict(rr[:, j, :, i, :],
                                  pt.rearrange("d (v pl) -> d v pl", v=V))

        ph = flags.get("phases", 99)
        for _lyr in (1, 2):
            if ph >= _lyr + 1 or _lyr == 1:
                with tc.tile_pool(name=f"attnp{_lyr}", bufs=1) as _ap:
                    attn_layer(_lyr, _ap, upto=("full" if ph >= _lyr + 1 else "a"))

        # ================= P3a: h1 = gelu(W_l1^T X2) =================
        bl1 = bias_sb.get("b_l1")
        if ph < 4:
            nc.sync.dma_start(out=out[0:P, :], in_=x1_hbm[0:P, :].bitcast(F32))
            reg_ctx.close()
            return nc
        with tc.tile_pool(name="f_w", bufs=2) as wp, \
             tc.tile_pool(name="f_st", bufs=2) as stp, \
             tc.tile_pool(name="f_ps", bufs=4, space="PSUM") as pp:
            for m in range(HT):
                wl_m = wp.tile([P, KT, P], F32R, name="wl_m")
                nc.sync.dma_start(
                    out=wl_m,
                    in_=w["w_l1"][:, m * P:(m + 1) * P].rearrange(
                        "(kt p) c -> p kt c", p=P))
                h1st = stp.tile([P, T], F32R, name="h1st")
                for n in range(NT):
                    ps_h = pp.tile([P, NCH], F32, name="ps_h")
                    for k in range(KT):
                        nc.tensor.matmul(
                            ps_h, wl_m[:, k, :],
                            reg[:, k, n * NCH:(n + 1) * NCH],
                            start=(k == 0), stop=(k == KT - 1))
                    nc.scalar.activation(
                        out=h1st[:, n * NCH:(n + 1) * NCH], in_=ps_h,
                        func=AF.Gelu,
                        bias=(bl1[:, m:m + 1] if bl1 is not None else 0.0),
                        scale=1.0)
                nc.gpsimd.dma_start(out=h1_hbm[m], in_=h1st)

        # region + attn tiles die here
        reg_ctx.close()
        if ph < 5:
            nc.sync.dma_start(out=out[0:P, :], in_=x2_hbm[0:P, :].bitcast(F32))
            return nc

        # ============ P3b/c: X3 = X2 + h1^T Wl2; LN3 -> out ============
        KG = 4
        g3_pair = (gain_sb.get("g3"), gain_sb.get("be3"))
        bl2 = bias_sb.get("b_l2")
        with tc.tile_pool(name="x3p", bufs=1) as x3p, \
             tc.tile_pool(name="h_in", bufs=2) as hp, \
             tc.tile_pool(name="w2_in", bufs=2) as w2p, \
             tc.tile_pool(name="x3ps", bufs=4, space="PSUM") as pp3, \
             tc.tile_pool(name="x3ln", bufs=4) as lnp3, \
             tc.tile_pool(name="x3out", bufs=1) as outp:
            pools["lnp"] = lnp3
            for halfb in range(2):
                x3 = x3p.tile([P, 8, PD], F32, name="x3")
                for t8 in range(8):
                    ti = halfb * 8 + t8
                    nc.sync.dma_start(
                        out=x3[:, t8, :],
                        in_=x2_hbm[ti * P:(ti + 1) * P, :].bitcast(F32))
                    if bl2 is not None:
                        nc.vector.tensor_add(x3[:, t8, :], x3[:, t8, :], bl2)
                for kg in range(HT // KG):
                    h1g = hp.tile([P, KG, T], F32R, name="h1g")
                    nc.sync.dma_start(
                        out=h1g,
                        in_=h1_hbm[kg * KG:(kg + 1) * KG].rearrange(
                            "m p t -> p m t"))
                    w2g = w2p.tile([P, KG, PD], F32R, name="w2g")
                    nc.sync.dma_start(
                        out=w2g,
                        in_=w["w_l2"][kg * KG * P:(kg + 1) * KG * P, :]
                        .rearrange("(m p) c -> p m c", p=P))
                    for t8 in range(8):
                        ti = halfb * 8 + t8
                        for f in range(NT):
                            ps3 = pp3.tile([P, NCH], F32, name="ps3")
                            for k in range(KG):
                                nc.tensor.matmul(
                                    ps3, h1g[:, k, ti * P:(ti + 1) * P],
                                    w2g[:, k, f * NCH:(f + 1) * NCH],
                                    start=(k == 0), stop=(k == KG - 1))
                            sl = slice(f * NCH, (f + 1) * NCH)
                            nc.vector.tensor_add(
                                x3[:, t8, sl], x3[:, t8, sl], ps3)
                for t8 in range(8):
                    ti = halfb * 8 + t8
                    ot = outp.tile([P, PD], F32, name="ot")
                    ln_norm(x3[:, t8, :], ot, g3_pair[0], g3_pair[1])
                    nc.gpsimd.dma_start(
                        out=out[ti * P:(ti + 1) * P, :], in_=ot)

    return nc


# ======================= host side =======================

_CACHE = {}


def _get_runner(flags):
    flags_key = (flags["use_bias"], flags["use_gains"], flags.get("debug", False), flags.get("phases", 99))
    if flags_key in _CACHE:
        return _CACHE[flags_key]
    import jax
    from jax.experimental.shard_map import shard_map
    from jax.sharding import Mesh, NamedSharding, PartitionSpec
    import concourse.mybir as mybir
    from concourse import bass2jax

    nc = build_encoder(flags)
    bass2jax.install_neuronx_cc_hook()
    if not nc.is_finalized():
        nc.finalize()

    partition_name = (
        nc.partition_id_tensor.name if nc.partition_id_tensor else None)
    in_names, out_names, out_avals, zero_outs = [], [], [], []
    for alloc in nc.m.functions[0].allocations:
        if not isinstance(alloc, mybir.MemoryLocationSet):
            continue
        name = alloc.memorylocations[0].name
        if alloc.kind == "ExternalInput":
            if name != partition_name:
                in_names.append(name)
        elif alloc.kind == "ExternalOutput":
            shape = tuple(alloc.tensor_shape)
            dtype = mybir.dt.np(alloc.dtype)
            out_names.append(name)
            out_avals.append(jax.core.ShapedArray(shape, dtype))
            zero_outs.append(np.zeros(shape, dtype))
    all_in_names = list(in_names) + list(out_names)
    if partition_name is not None:
        all_in_names.append(partition_name)

    def _body(*args):
        operands = list(args)
        if partition_name is not None:
            operands.append(bass2jax.partition_id_tensor())
        outs = bass2jax._bass_exec_p.bind(
            *operands,
            out_avals=tuple(out_avals),
            in_names=tuple(all_in_names),
            out_names=tuple(out_names),
            lowering_input_output_aliases=(),
            sim_require_finite=False,
            sim_require_nnan=False,
            nc=nc)
        return tuple(outs)

    devices = jax.devices()[:B]
    mesh = Mesh(np.asarray(devices), ("core",))
    spec = PartitionSpec("core")
    sharding = NamedSharding(mesh, spec)
    n_io = len(in_names) + len(out_names)
    fn = jax.jit(
        shard_map(_body, mesh=mesh, in_specs=(spec,) * n_io,
                  out_specs=(spec,) * len(out_names), check_rep=False),
        keep_unused=True)
    r = {"fn": fn, "in_names": in_names, "out_names": out_names,
         "zero_outs": zero_outs, "sharding": sharding}
    _CACHE[flags_key] = r
    return r


def prepare_inputs(inputs):
    """Full inputs -> (flags, per-core input maps)."""
    x = np.ascontiguousarray(
        np.asarray(inputs["x"], dtype=np.float32).reshape(B, T, PD))
    var_mask = np.asarray(inputs["var_mask"])

    def z(name):
        return not np.any(np.asarray(inputs[name]))

    use_bias = not all(z(n) for n in (
        "b_q1", "b_kv1", "b_o1", "b_q2", "b_kv2", "b_o2", "b_l1", "b_l2"))
    g_triv = (np.all(np.asarray(inputs["g1"]) == 1.0)
              and np.all(np.asarray(inputs["g2"]) == 1.0)
              and np.all(np.asarray(inputs["g3"]) == 1.0)
              and z("be1") and z("be2") and z("be3"))
    flags = {"use_bias": use_bias, "use_gains": not g_triv}

    pl_neq = (np.arange(GP)[:, None] != np.arange(GP)[None, :])
    weights = {}
    for n in ("w_q1", "w_kv1", "w_o1", "w_q2", "w_kv2", "w_o2",
              "w_l1", "w_l2"):
        weights[n] = np.ascontiguousarray(np.asarray(inputs[n], np.float32))
    in_maps = []
    for b in range(B):
        m4 = np.where(
            var_mask[b][:, None, :, None] | pl_neq[None, :, None, :],
            np.float32(NEG), np.float32(0.0))            # [v, pl, r, pl2]
        im = {"x": x[b],
              "mask2": np.ascontiguousarray(m4.reshape(P, P))}
        im.update(weights)
        if use_bias:
            for n in ("b_q1", "b_kv1", "b_o1", "b_q2", "b_kv2", "b_o2",
                      "b_l1", "b_l2"):
                im[n] = np.ascontiguousarray(np.asarray(inputs[n], np.float32))
        if flags["use_gains"]:
            for n in ("g1", "be1", "g2", "be2", "g3", "be3"):
                im[n] = np.ascontiguousarray(np.asarray(inputs[n], np.float32))
        in_maps.append(im)
    return flags, in_maps


def run_on_device(flags, in_maps, time_iters=0):
    import jax
    r = _get_runner(flags)
    concat = [np.concatenate([m[n] for m in in_maps], axis=0)
              for n in r["in_names"]]
    concat += [np.zeros((B * z.shape[0], *z.shape[1:]), z.dtype)
               for z in r["zero_outs"]]
    dev_in = [jax.device_put(a, r["sharding"]) for a in concat]
    jax.block_until_ready(dev_in)
    outs = r["fn"](*dev_in)
    jax.block_until_ready(outs)
    burst = None
    if time_iters:
        t0 = time.perf_counter()
        for _ in range(time_iters):
            outs = r["fn"](*dev_in)
        jax.block_until_ready(outs)
        burst = (time.perf_counter() - t0) / time_iters
    out_full = np.asarray(outs[r["out_names"].index("out")])
    return out_full.reshape(B, T, PD), burst


def kernel(**inputs):
    flags, in_maps = prepare_inputs(inputs)
    out, _ = run_on_device(flags, in_maps)
    x_out = out.reshape(B, V, P, PD).astype(np.float32)
    x_next = x_out.reshape(B, V, P // 2, 2 * L, D)
    return x_out, x_next
